# revision 40
# baseline (speedup 1.0000x reference)
"""Trainium2 Bass kernel for the (misordered-scale) MultiHeadAttention problem.

Problem (per batch b of 8, one NeuronCore each):
  qk = x @ Wqk.T + bqk            # [2048, 512], channel c = 2*(h*64+e) + {0:q, 1:k}
  v  = x @ Wv.T  + bv             # [2048, 256], channel c = h*64+e
  S_h = q_h @ k_h.T               # [2048, 2048] per head (e=64)
  attn = softmax(S, -1) / 16
  out_h = attn_h @ v_h            # [2048, 64]
  out = concat_h(out_h) @ Wo.T + bo   # [2048, 1024]

Strategy: data-parallel over batch across 8 cores (no collectives).

Schedule (single dense PE stream, cost-model-guided):
  - x is cast fp32->fp16 on the Pool SWDGE ring (256-token chunks) and
    transposed into SBUF by the SP-ring xbar; four rotating DRAM stages
    pace the casts against the transposes via real WAR deps.  Wq/Wk/Wv
    load as fp32 rows on the ACT ring and are transposed on the PE
    (identity matmuls), filling the DMA-bound prologue.
  - Projections run as 8-matmul accumulation chains on a 3-deep shared
    PSUM rotation; q(ib=0) + all k + all v run up front (x-arrival
    order), q(ib=1..3) chains are split into 4-matmul filler quanta
    issued inside the attention phase.
  - Attention per sweep (ib, head-pair): S in the classic [j, i]
    orientation (out [128, 1024] per j-block pair of heads), exp on ACT
    (fused bias -8, bf16 out), but AV runs TRANSPOSED: per (head,
    i128-block, j-block) one matmul with lhsT = the exp tile slice
    (stationary) and rhs = [v_h | 16.0], accumulating out[i, 0:65] in
    PSUM.  Out free size is 65, so the whole AV phase costs ~1/8 of the
    classic orientation on the PE.  Column 64 accumulates 16*sum(exp),
    folding the 1/16 mis-scale into the denominator for free.
  - Normalization is a per-partition scalar multiply: in the [i, e]
    layout the softmax denominator is one column, so DVE reciprocal +
    tensor_scalar (fp32 rec) normalizes and casts to fp16 in one op per
    (head, i128).  PE transposes (identity matmuls, fp16) restore the
    [c, i] layout for the out-projection; they are deferred into the
    next sweep's filler queue.
  - Wo stays fp16 (cast + transpose DMA); out tiles are evacuated with the
    bo broadcast on DVE and stored straight from SBUF.
"""

import numpy as np
from contextlib import ExitStack

import concourse.bass as bass
import concourse.mybir as mybir
import concourse.tile as tile
from concourse import bacc
from concourse import bass_utils

FP32 = mybir.dt.float32
BF16 = mybir.dt.bfloat16
FP16 = mybir.dt.float16
AF = mybir.ActivationFunctionType
ALU = mybir.AluOpType

B = 8
N = 2048          # tokens per batch
D = 1024          # model dim
H = 4             # heads
E = 64            # per-head dim after the einops split
HD = 256          # H*E (v channels / Wo contraction dim)
NCORES = 8

DC = D // 128      # 8 d-chunks of 128
NIB = N // 512     # 4 i-blocks of 512
NJB = N // 128     # 16 j-blocks of 128
# exp(S - 8): headroom offset for the exp path (max logit ~51 -> e^43 fits
# bf16 comfortably); the offset cancels exactly in the softmax normalization.
EXP_BIAS = -8.0
# AV ones-column value: makes the av column 64 equal 16*sum(exp), so its
# reciprocal is directly the softmax/16 normalization factor.
DEN_SCALE = 16.0

N_WARM = 14        # PE warmup matmuls (keep PE busy until first chain is fed)
N_STAGES = 4       # rotating DRAM staging tiles for the x supply chain
X_CHUNK = 256      # tokens per x supply chunk
EXP_BUFS = 4       # SBUF exp tile ring depth
FILL_EVERY = 2     # jb period between filler slots in a sweep
NT_POS = 8         # yt-tail position in the next sweep's filler queue
WO_DEPRIO = True   # schedule the Wo cast only when the Pool ring is idle


def _build_kernel(nc: bass.Bass, tc: tile.TileContext, out_ap, x, wqk, bqk, wv, bv, wo, bo):
    ctx = ExitStack()
    with ctx:
        consts = ctx.enter_context(tc.tile_pool(name="consts", bufs=1))
        dram = ctx.enter_context(tc.tile_pool(name="dram", bufs=1, space="DRAM"))
        exps_pool = ctx.enter_context(tc.tile_pool(name="exps", bufs=EXP_BUFS))
        osb_pool = ctx.enter_context(tc.tile_pool(name="osb", bufs=6))
        avn_pool = ctx.enter_context(tc.tile_pool(name="avn", bufs=2))
        ps = ctx.enter_context(tc.tile_pool(name="ps", bufs=3, space="PSUM"))
        psav = ctx.enter_context(tc.tile_pool(name="psav", bufs=1, space="PSUM"))

        # ---------------- persistent SBUF tensors ----------------
        # Internal d layout (chunk dc at partition p holds d = dc*128 + p) is
        # shared by xt (xbar transpose) and the W^T tiles (PE transpose), so
        # d stays purely internal to the contractions.
        xt = consts.tile([128, DC, N], FP16)          # x^T
        wqT = consts.tile([128, DC, HD], FP16)        # Wq^T: [d, c]
        wkT = consts.tile([128, DC, HD], FP16)
        wvT = consts.tile([128, DC, HD], FP16)
        wq32 = consts.tile([128, 2, D], FP32)         # Wq rows (c-major), fp32
        wk32 = consts.tile([128, 2, D], FP32)
        wv32 = consts.tile([128, 2, D], FP32)
        ident = consts.tile([128, 128], FP32)         # PE-transpose identity
        identh = consts.tile([128, 128], FP16)        # fp16 identity (avn transposes)
        it_f = consts.tile([128, 128], FP32)
        it_p = consts.tile([128, 1], FP32)
        woT = consts.tile([128, 2, D], FP16)          # Wo^T: [c, do]
        qT = consts.tile([128, 2, N], FP16)           # q^T: [c, i]; c = cc*128 + p
        kT = consts.tile([128, 2, N], FP16)
        yT = consts.tile([128, 2, N], FP16)           # concat-head attn out, feature-major
        vh = consts.tile([128, NJB, H, 66], BF16)     # [j, jb, h, 0:64]=v_h, [...,64]=16.0
        bq_sb = consts.tile([128, 2, 1], FP32)        # q bias per partition (c)
        bk_sb = consts.tile([128, 2, 1], FP32)
        bv_row = consts.tile([1, HD], FP32)           # v bias as K=1 matmul rhs
        bo_row = consts.tile([1, D], FP32)
        ones32 = consts.tile([1, 128], FP32)
        wdum = consts.tile([1, 128], FP16)            # warmup stationary
        rdum = consts.tile([1, 512], FP16)            # warmup moving
        expb = consts.tile([128, 1], FP32)
        vbc = consts.tile([128, HD], FP32)            # bv broadcast over tokens
        obc = consts.tile([128, 2, 512], FP32)        # bo broadcast over tokens
        nc.vector.memset(ones32[:], 1.0)
        nc.vector.memset(wdum[:], 0.0)
        nc.vector.memset(rdum[:], 0.0)
        # identity for PE transposes: ident[p, f] = (f == p)
        nc.gpsimd.iota(it_f[:], [[0, 1], [1, 128]], base=0, channel_multiplier=0,
                       allow_small_or_imprecise_dtypes=True)
        nc.gpsimd.iota(it_p[:], [[0, 1], [1, 1]], base=0, channel_multiplier=1,
                       allow_small_or_imprecise_dtypes=True)
        nc.vector.tensor_scalar(ident[:], it_f[:], it_p[:], None, ALU.is_equal)
        nc.vector.tensor_copy(identh[:], ident[:])
        nc.vector.memset(expb[:], EXP_BIAS)
        nc.vector.memset(vh[:, :, :, 64:66], DEN_SCALE)

        # ---------------- DRAM staging (fp16 casts) ----------------
        # One DRAM tile per transfer: the tile scheduler tracks DRAM tiles at
        # whole-tile granularity, so a shared staging buffer would serialize
        # every cast against every transpose.  Rotating 256-token x stages
        # throttle the cast stream via real WAR dependencies -- cast i must
        # wait for transpose i-4 -- which forces the scheduler to interleave
        # transposes with casts on the shared DMA device instead of running
        # every cast first.
        xstg = [dram.tile([X_CHUNK, D], FP16, name=f"xst{i}") for i in range(N_STAGES)]
        wo_bf = dram.tile([D, HD], FP16)

        # De-interleave Wqk rows: q rows are 2c, k rows are 2c+1.
        wqk_r = wqk.rearrange("(c s) d -> s c d", s=2)
        bqk_r = bqk.rearrange("(c s) -> s c", s=2)

        # bias loads ride the ACT HWDGE ring (cheap, off the SP transpose ring)
        for cb in range(2):
            nc.scalar.dma_start(bq_sb[:, cb, :], bqk_r[0, cb * 128:(cb + 1) * 128])
            nc.scalar.dma_start(bk_sb[:, cb, :], bqk_r[1, cb * 128:(cb + 1) * 128])
        nc.scalar.dma_start(bv_row[0:1, :], bv[:])
        nc.scalar.dma_start(bo_row[0:1, :], bo[:])

        # ---------------- supply: x casts + xbar transposes, W direct -------
        def x_sup(ci):
            t0, t1 = ci * X_CHUNK, (ci + 1) * X_CHUNK
            st = xstg[ci % N_STAGES]
            nc.gpsimd.dma_start(st[:, :], x[t0:t1, :])
            nc.sync.dma_start(xt[:, :, t0:t1], st[:, :], transpose=True)

        nch = N // X_CHUNK
        x_sup(0)
        nc.scalar.dma_start(wq32[:, 0, :], wqk_r[0, 0:128])
        nc.scalar.dma_start(wq32[:, 1, :], wqk_r[0, 128:256])
        x_sup(1)
        nc.scalar.dma_start(wk32[:, 0, :], wqk_r[1, 0:128])
        nc.scalar.dma_start(wk32[:, 1, :], wqk_r[1, 128:256])
        if nch > 4:
            x_sup(2)
            x_sup(3)
        nc.scalar.dma_start(wv32[:, 0, :], wv[0:128, :])
        nc.scalar.dma_start(wv32[:, 1, :], wv[128:256, :])
        for ci in range(min(4, nch // 2), nch):
            x_sup(ci)
        prio0 = tc.cur_priority
        if WO_DEPRIO:
            # Make the Wo supply look late to the tile scheduler so it cannot
            # jump the shared DMA device ahead of the x/k/v supply.
            tc.cur_priority += 100000
        nc.gpsimd.dma_start(wo_bf[:], wo[:])
        for g in range(2):
            # Wo^T must match yT's c-layout (c = cc*128 + p): per-chunk.
            cs = slice(g * 128, (g + 1) * 128)
            nc.sync.dma_start(woT[:, g, :], wo_bf[:, cs], transpose=True)
        if WO_DEPRIO:
            tc.cur_priority = prio0

        # ---------------- PE warmup ----------------
        # Keep the PE stream dense from ~1us so the p-state ramp is spent on
        # throwaway work, and the first real chains run at full clock.
        for _ in range(N_WARM):
            pw = ps.tile([128, 512], FP32, tag="sp")
            nc.tensor.matmul(pw[:], lhsT=wdum[:], rhs=rdum[:],
                             start=True, stop=True)

        def bias_broadcasts():
            pb = ps.tile([128, 512], FP32, tag="sp")
            nc.tensor.matmul(pb[:, 0:HD], lhsT=ones32[:], rhs=bv_row[:],
                             start=True, stop=True)
            nc.vector.tensor_copy(vbc[:], pb[:, 0:HD])
            for ob in range(2):
                pb2 = ps.tile([128, 512], FP32, tag="sp")
                nc.tensor.matmul(pb2[:], lhsT=ones32[:],
                                 rhs=bo_row[:, ob * 512:(ob + 1) * 512],
                                 start=True, stop=True)
                nc.vector.tensor_copy(obc[:, ob, :], pb2[:])

        # ---------------- W^T via PE transpose ----------------
        # 4 transposed [128,128] d-chunks per PSUM slab, one strided DVE
        # evacuation (with the fp16 cast) per slab.
        def w_transpose(w32, dstT):
            for cb in range(2):
                for g in range(2):
                    tp = ps.tile([128, 512], FP32, tag="sp")
                    for j in range(4):
                        dc = g * 4 + j
                        nc.tensor.transpose(
                            tp[:, j * 128:(j + 1) * 128],
                            w32[:, cb, dc * 128:(dc + 1) * 128],
                            ident[:],
                        )
                    nc.vector.tensor_copy(
                        dstT[:, g * 4:(g + 1) * 4, cb * 128:(cb + 1) * 128],
                        tp[:].rearrange("p (dc c) -> p dc c", dc=4),
                    )

        # ---------------- projection chains ----------------
        def qk_chain(wT, b_sb, dstT, cb, t0, t1):
            pp = ps.tile([128, t1 - t0], FP32, tag="sp")
            for dc in range(DC):
                nc.tensor.matmul(
                    pp[:],
                    lhsT=wT[:, dc, cb * 128:(cb + 1) * 128],
                    rhs=xt[:, dc, t0:t1],
                    start=(dc == 0),
                    stop=(dc == DC - 1),
                )
            # evacuate + per-partition bias + cast to fp16 on DVE
            nc.vector.tensor_scalar(
                dstT[:, cb, t0:t1], pp[:], b_sb[:, cb, :], None, ALU.add,
            )

        def v_chain(jb):
            pv = ps.tile([128, 512], FP32, tag="sp")
            for dc in range(DC):
                nc.tensor.matmul(
                    pv[:, 0:HD],
                    lhsT=xt[:, dc, jb * 128:(jb + 1) * 128],
                    rhs=wvT[:, dc, :],
                    start=(dc == 0),
                    stop=(dc == DC - 1),
                )
            nc.vector.tensor_tensor(vh[:, jb, :, 0:64], pv[:, 0:HD], vbc[:], ALU.add)

        # ---------------- attention sweep machinery ----------------
        # One sweep = (ib, cc): i-block of 512 tokens x head pair (2cc, 2cc+1).
        # S is the classic [j, i] orientation; AV is transposed: av[i, 0:65]
        # accumulates over all 16 j-blocks with the exp tile as the stationary
        # operand, so each AV matmul is charged only 65 output columns.
        def make_sweep(cc, isl):
            avA = psav.tile([128, 4, 128], FP32, tag="avA", bufs=1, name="avA")
            avB = psav.tile([128, 4, 128], FP32, tag="avB", bufs=1, name="avB")
            st = {"avA": avA, "avB": avB, "exs": {}}

            def emit_S(jb):
                jsl = slice(jb * 128, (jb + 1) * 128)
                sp = ps.tile([128, 1024], FP32, tag="sp")
                nc.tensor.matmul(
                    sp[:, 0:512],
                    lhsT=kT[0:64, cc, jsl], rhs=qT[0:64, cc, isl],
                    start=True, stop=True,
                )
                nc.tensor.matmul(
                    sp[:, 512:1024],
                    lhsT=kT[64:128, cc, jsl], rhs=qT[64:128, cc, isl],
                    start=True, stop=True,
                )
                ex = exps_pool.tile([128, 1024], BF16)
                nc.scalar.activation(ex[:], sp[:], AF.Exp, bias=expb[:])
                st["exs"][jb] = ex

            def emit_AV(jb):
                ex = st["exs"].pop(jb)
                last = (jb == NJB - 1)
                for t in range(4):
                    tsl = slice(t * 128, (t + 1) * 128)
                    # start=True clears has_written for the WHOLE PSUM bank,
                    # so only the first region of the first j-block may carry
                    # it; the other t-regions of jb 0 write onto the cleared
                    # bank with start=False (has_written=0 -> plain write).
                    first = (jb == 0) and (t == 0)
                    nc.tensor.matmul(
                        st["avA"][:, t, 0:65],
                        lhsT=ex[:, tsl], rhs=vh[:, jb, 2 * cc, 0:65],
                        start=first, stop=last, skip_group_check=True,
                    )
                    nc.tensor.matmul(
                        st["avB"][:, t, 0:65],
                        lhsT=ex[:, 512 + t * 128:512 + (t + 1) * 128],
                        rhs=vh[:, jb, 2 * cc + 1, 0:65],
                        start=first, stop=last, skip_group_check=True,
                    )

            st["S"] = emit_S
            st["AV"] = emit_AV
            return st

        # Normalize at sweep end (DVE): rec = 1/(16*sum exp) is one column of
        # the transposed av, so a per-partition reciprocal + tensor_scalar
        # multiply normalizes and casts to fp16 in one op per (head, i128).
        def normalize(st):
            rec = avn_pool.tile([128, 4, 2], FP32, tag="rec")
            nc.vector.reciprocal(rec[:, :, 0:1], st["avA"][:, :, 64:65])
            nc.vector.reciprocal(rec[:, :, 1:2], st["avB"][:, :, 64:65])
            avnA = avn_pool.tile([128, 4, E], FP16, tag="avnA")
            avnB = avn_pool.tile([128, 4, E], FP16, tag="avnB")
            for t in range(4):
                nc.vector.tensor_scalar(
                    avnA[:, t, :], st["avA"][:, t, 0:E], rec[:, t, 0:1],
                    None, ALU.mult,
                )
                nc.vector.tensor_scalar(
                    avnB[:, t, :], st["avB"][:, t, 0:E], rec[:, t, 1:2],
                    None, ALU.mult,
                )
            return avnA, avnB

        # Deferred into the next sweep's fillers: PE transposes restore the
        # [c, i] layout for the out-projection (head pair stacked on
        # partitions, two per [128, 128] PSUM tile), one DVE copy each.
        def make_yt_tail(cc, ib, avnA, avnB, t):
            def yt_tail():
                tp = ps.tile([128, 128], FP16, tag="sp")
                nc.tensor.transpose(tp[0:64, :], avnA[:, t, :], identh[:])
                nc.tensor.transpose(tp[64:128, :], avnB[:, t, :], identh[:])
                isl = slice(ib * 512 + t * 128, ib * 512 + (t + 1) * 128)
                nc.vector.tensor_copy(yT[:, cc, isl], tp[:])
            return yt_tail

        # Sweep (ib0, cc0) state: its S/exp/AV stream is interleaved into the
        # projection phase below so the ACT exp pipe starts ~40us earlier.
        s00 = make_sweep(0, slice(0, 512))

        def s00_block(b):
            for jb in range(4 * b, 4 * b + 4):
                s00["S"](jb)
                if jb >= 2:
                    s00["AV"](jb - 2)

        # W transposes and q(ib=0)/k/v chains in x-arrival order; the Wk/Wv
        # fp32 loads are emitted at their need sites so the tile scheduler
        # cannot let them jump the DMA device ahead of the early x chunks.
        w_transpose(wq32, wqT)
        for t0, t1 in ((0, 256), (256, 512)):
            qk_chain(wqT, bq_sb, qT, 0, t0, t1)
            qk_chain(wqT, bq_sb, qT, 1, t0, t1)
        w_transpose(wk32, wkT)
        qk_chain(wkT, bk_sb, kT, 0, 0, 512)
        qk_chain(wkT, bk_sb, kT, 1, 0, 512)
        bias_broadcasts()
        w_transpose(wv32, wvT)
        for jb in range(4):
            v_chain(jb)
        s00_block(0)
        for tb in range(1, 4):
            ts0, ts1 = tb * 512, (tb + 1) * 512
            qk_chain(wkT, bk_sb, kT, 0, ts0, ts1)
            qk_chain(wkT, bk_sb, kT, 1, ts0, ts1)
            for jb in range(tb * 4, (tb + 1) * 4):
                v_chain(jb)
            s00_block(tb)

        # ---------------- attention + out-projection ----------------
        def oproj_tile(it, ob):
            # out[i, do] = sum_c yT[c, i] * WoT[c, do] + bo[do]
            tsl = slice(it * 128, (it + 1) * 128)
            po = ps.tile([128, 512], FP32, tag="sp")
            osl = slice(ob * 512, (ob + 1) * 512)
            for cc2 in range(2):
                nc.tensor.matmul(
                    po[:],
                    lhsT=yT[:, cc2, tsl],
                    rhs=woT[:, cc2, osl],
                    start=(cc2 == 0), stop=(cc2 == 1),
                )
            osb = osb_pool.tile([128, 512], FP32)
            nc.vector.tensor_tensor(osb[:], po[:], obc[:, ob, :], ALU.add)
            nc.sync.dma_start(out_ap[tsl, osl], osb[:])

        # q-chains are split into two ~4-matmul filler quanta so they slot
        # between S pairs without draining the exp stream.  Parts A and B of
        # one chain must land on consecutive filler slots: with 3 sp slabs
        # the chain's accumulator survives exactly two interleaved S-pair
        # allocations.
        def q_fill_pair(ib2, cb):
            t0, t1 = ib2 * 512, (ib2 + 1) * 512
            state = {}

            def part_a():
                pp = ps.tile([128, 512], FP32, tag="sp", name="qf")
                state["pp"] = pp
                for dc in range(4):
                    nc.tensor.matmul(
                        pp[:],
                        lhsT=wqT[:, dc, cb * 128:(cb + 1) * 128],
                        rhs=xt[:, dc, t0:t1],
                        start=(dc == 0), stop=False,
                    )

            def part_b():
                pp = state["pp"]
                for dc in range(4, DC):
                    nc.tensor.matmul(
                        pp[:],
                        lhsT=wqT[:, dc, cb * 128:(cb + 1) * 128],
                        rhs=xt[:, dc, t0:t1],
                        start=False, stop=(dc == DC - 1),
                    )
                nc.vector.tensor_scalar(
                    qT[:, cb, t0:t1], pp[:], bq_sb[:, cb, :], None, ALU.add,
                )
            return [part_a, part_b]

        # Per-sweep filler queues (sweep s = ib*2 + cc).  The deferred yt
        # tails of sweep s-1 are inserted inside the loop.  Sweep 0's S/exp/AV
        # work is interleaved into the projection phase (it has no filler
        # slots), so the q-chain fillers all land in sweeps 1-5.
        sweep_fillers = {s: [] for s in range(2 * NIB + 1)}
        sweep_fillers[1] = q_fill_pair(1, 0) + q_fill_pair(1, 1)
        sweep_fillers[2] = q_fill_pair(2, 0)
        sweep_fillers[3] = q_fill_pair(2, 1)
        sweep_fillers[4] = q_fill_pair(3, 0)
        sweep_fillers[5] = q_fill_pair(3, 1)
        for ib in range(1, NIB):
            prev = ib - 1
            tiles = [(prev * 4 + t, o) for t in range(4) for o in range(2)]
            for k in range(2):
                sweep_fillers[2 * ib + k] += [
                    (lambda it=it, ob=ob: oproj_tile(it, ob))
                    for it, ob in tiles[k * 4:(k + 1) * 4]
                ]
        sweep_fillers[2 * NIB] = [
            (lambda it=12 + t, ob=o: oproj_tile(it, ob))
            for t in range(4) for o in range(2)
        ]

        for ib in range(NIB):
            isl = slice(ib * 512, (ib + 1) * 512)
            for cc in range(2):          # head pair (2*cc, 2*cc+1)
                s_idx = ib * 2 + cc
                fl = sweep_fillers[s_idx]
                fi = 0
                if s_idx == 0:
                    # S/exp and AV(0..13) were interleaved into the
                    # projection phase; only the pipeline drain remains.
                    st = s00
                    st["AV"](NJB - 2)
                    st["AV"](NJB - 1)
                else:
                    st = make_sweep(cc, isl)
                    # software pipeline: S runs two j-blocks ahead of AV, so
                    # each exp has a full extra period of ACT margin.
                    st["S"](0)
                    st["S"](1)
                    for jb in range(2, NJB):
                        st["S"](jb)
                        st["AV"](jb - 2)
                        if jb % FILL_EVERY == 1 and fi < len(fl):
                            fl[fi]()
                            fi += 1
                    st["AV"](NJB - 2)
                    st["AV"](NJB - 1)
                while fi < len(fl):
                    fl[fi]()
                    fi += 1

                avnA, avnB = normalize(st)
                tails = [make_yt_tail(cc, ib, avnA, avnB, t) for t in range(4)]
                nxt = sweep_fillers[s_idx + 1]
                pos = min(NT_POS, len(nxt))
                nxt[pos:pos] = tails

        # tail: last yt transposes first, then out-projection of the last
        # i-block.  Data deps (semaphores) order the yT writes before their
        # oproj readers.
        tail = sweep_fillers[2 * NIB]
        for f in tail:
            f()


_CACHE: dict = {}


def _get_compiled():
    key = "nc"
    if key in _CACHE:
        return _CACHE[key]
    nc = bacc.Bacc(
        "TRN2", target_bir_lowering=False, debug=False, num_devices=NCORES,
    )
    x = nc.dram_tensor("x", (N, D), FP32, kind="ExternalInput").ap()
    wqk = nc.dram_tensor("Wqk", (2 * HD, D), FP32, kind="ExternalInput").ap()
    bqk = nc.dram_tensor("bqk", (2 * HD,), FP32, kind="ExternalInput").ap()
    wv = nc.dram_tensor("Wv", (HD, D), FP32, kind="ExternalInput").ap()
    bv = nc.dram_tensor("bv", (HD,), FP32, kind="ExternalInput").ap()
    wo = nc.dram_tensor("Wo", (D, HD), FP32, kind="ExternalInput").ap()
    bo = nc.dram_tensor("bo", (D,), FP32, kind="ExternalInput").ap()
    out = nc.dram_tensor("out", (N, D), FP32, kind="ExternalOutput").ap()

    with tile.TileContext(nc) as tc:
        _build_kernel(nc, tc, out, x, wqk, bqk, wv, bv, wo, bo)
    nc.compile()
    _CACHE[key] = nc
    return nc


def run_cores(in_maps, trace=False, **kw):
    nc = _get_compiled()
    return bass_utils.run_bass_kernel_spmd(
        nc, in_maps, core_ids=list(range(NCORES)), trace=trace, **kw
    )


def kernel(x, Wqk, bqk, Wv, bv, Wo, bo):
    x = np.asarray(x, dtype=np.float32)
    in_maps = [
        {
            "x": np.ascontiguousarray(x[c]),
            "Wqk": np.asarray(Wqk, np.float32),
            "bqk": np.asarray(bqk, np.float32),
            "Wv": np.asarray(Wv, np.float32),
            "bv": np.asarray(bv, np.float32),
            "Wo": np.asarray(Wo, np.float32),
            "bo": np.asarray(bo, np.float32),
        }
        for c in range(NCORES)
    ]
    # The axon tunnel occasionally returns a glitched execution (transient
    # non-finite garbage); retry a couple of times in that case.
    for _attempt in range(3):
        res = run_cores(in_maps)
        out = np.stack([res.results[c]["out"] for c in range(NCORES)], axis=0)
        if np.isfinite(out).all():
            break
    return out


# revision 44
# speedup vs baseline: 1.0073x; 1.0073x over previous
"""Trainium2 Bass kernel for the (misordered-scale) MultiHeadAttention problem.

Problem (per batch b of 8, one NeuronCore each):
  qk = x @ Wqk.T + bqk            # [2048, 512], channel c = 2*(h*64+e) + {0:q, 1:k}
  v  = x @ Wv.T  + bv             # [2048, 256], channel c = h*64+e
  S_h = q_h @ k_h.T               # [2048, 2048] per head (e=64)
  attn = softmax(S, -1) / 16
  out_h = attn_h @ v_h            # [2048, 64]
  out = concat_h(out_h) @ Wo.T + bo   # [2048, 1024]

Strategy: data-parallel over batch across 8 cores (no collectives).

Schedule (single dense PE stream, cost-model-guided):
  - x is cast fp32->fp16 on the Pool SWDGE ring (256-token chunks) and
    transposed into SBUF by the SP-ring xbar; four rotating DRAM stages
    pace the casts against the transposes via real WAR deps.  Wq/Wk/Wv
    load as fp32 rows on the ACT ring and are transposed on the PE
    (identity matmuls), filling the DMA-bound prologue.
  - Projections run as 8-matmul accumulation chains on a 3-deep shared
    PSUM rotation; q(ib=0) + all k + all v run up front (x-arrival
    order), q(ib=1..3) chains are split into 4-matmul filler quanta
    issued inside the attention phase.
  - Attention per sweep (ib, head-pair): S in the classic [j, i]
    orientation (out [128, 1024] per j-block pair of heads), exp on ACT
    (fused bias -8, bf16 out), but AV runs TRANSPOSED: per (head,
    i128-block, j-block) one matmul with lhsT = the exp tile slice
    (stationary) and rhs = [v_h | 16.0], accumulating out[i, 0:65] in
    PSUM.  Out free size is 65, so the whole AV phase costs ~1/8 of the
    classic orientation on the PE.  Column 64 accumulates 16*sum(exp),
    folding the 1/16 mis-scale into the denominator for free.
  - Normalization is a per-partition scalar multiply: in the [i, e]
    layout the softmax denominator is one column, so DVE reciprocal +
    tensor_scalar (fp32 rec) normalizes and casts to fp16 in one op per
    (head, i128).  PE transposes (identity matmuls, fp16) restore the
    [c, i] layout for the out-projection; they are deferred into the
    next sweep's filler queue.
  - Wo stays fp16 (cast + transpose DMA); out tiles are evacuated with the
    bo broadcast on DVE and stored straight from SBUF.
"""

import numpy as np
from contextlib import ExitStack

import concourse.bass as bass
import concourse.mybir as mybir
import concourse.tile as tile
from concourse import bacc
from concourse import bass_utils

FP32 = mybir.dt.float32
BF16 = mybir.dt.bfloat16
FP16 = mybir.dt.float16
AF = mybir.ActivationFunctionType
ALU = mybir.AluOpType

B = 8
N = 2048          # tokens per batch
D = 1024          # model dim
H = 4             # heads
E = 64            # per-head dim after the einops split
HD = 256          # H*E (v channels / Wo contraction dim)
NCORES = 8

DC = D // 128      # 8 d-chunks of 128
NIB = N // 512     # 4 i-blocks of 512
NJB = N // 128     # 16 j-blocks of 128
# exp(S - 8): headroom offset for the exp path (max logit ~51 -> e^43 fits
# bf16 comfortably); the offset cancels exactly in the softmax normalization.
EXP_BIAS = -8.0
# AV ones-column value: makes the av column 64 equal 16*sum(exp), so its
# reciprocal is directly the softmax/16 normalization factor.
DEN_SCALE = 16.0

N_WARM = 14        # PE warmup matmuls (keep PE busy until first chain is fed)
N_STAGES = 4       # rotating DRAM staging tiles for the x supply chain
X_CHUNK = 256      # tokens per x supply chunk
EXP_BUFS = 4       # SBUF exp tile ring depth
FILL_EVERY = 2     # jb period between filler slots in a sweep
NT_POS = 8         # yt-tail position in the next sweep's filler queue
WO_DEPRIO = True   # schedule the Wo cast only when the Pool ring is idle


def _build_kernel(nc: bass.Bass, tc: tile.TileContext, out_ap, x, wqk, bqk, wv, bv, wo, bo):
    ctx = ExitStack()
    with ctx:
        consts = ctx.enter_context(tc.tile_pool(name="consts", bufs=1))
        dram = ctx.enter_context(tc.tile_pool(name="dram", bufs=1, space="DRAM"))
        exps_pool = ctx.enter_context(tc.tile_pool(name="exps", bufs=EXP_BUFS))
        osb_pool = ctx.enter_context(tc.tile_pool(name="osb", bufs=6))
        avn_pool = ctx.enter_context(tc.tile_pool(name="avn", bufs=2))
        ps = ctx.enter_context(tc.tile_pool(name="ps", bufs=3, space="PSUM"))
        psav = ctx.enter_context(tc.tile_pool(name="psav", bufs=1, space="PSUM"))

        # ---------------- persistent SBUF tensors ----------------
        # Internal d layout (chunk dc at partition p holds d = dc*128 + p) is
        # shared by xt (xbar transpose) and the W^T tiles (PE transpose), so
        # d stays purely internal to the contractions.
        xt = consts.tile([128, DC, N], FP16)          # x^T
        wqT = consts.tile([128, DC, HD], FP16)        # Wq^T: [d, c]
        wkT = consts.tile([128, DC, HD], FP16)
        wvT = consts.tile([128, DC, HD], FP16)
        wq32 = consts.tile([128, 2, D], FP32)         # Wq rows (c-major), fp32
        wk32 = consts.tile([128, 2, D], FP32)
        wv32 = consts.tile([128, 2, D], FP32)
        ident = consts.tile([128, 128], FP32)         # PE-transpose identity
        identh = consts.tile([128, 128], FP16)        # fp16 identity (avn transposes)
        it_f = consts.tile([128, 128], FP32)
        it_p = consts.tile([128, 1], FP32)
        woT = consts.tile([128, 2, D], FP16)          # Wo^T: [c, do]
        qT = consts.tile([128, 2, N], FP16)           # q^T: [c, i]; c = cc*128 + p
        kT = consts.tile([128, 2, N], FP16)
        yT = consts.tile([128, 2, N], FP16)           # concat-head attn out, feature-major
        vh = consts.tile([128, NJB, H, 66], BF16)     # [j, jb, h, 0:64]=v_h, [...,64]=16.0
        bq_sb = consts.tile([128, 2, 1], FP32)        # q bias per partition (c)
        bk_sb = consts.tile([128, 2, 1], FP32)
        bv_row = consts.tile([1, HD], FP32)           # v bias as K=1 matmul rhs
        bo_row = consts.tile([1, D], FP32)
        ones32 = consts.tile([1, 128], FP32)
        onesb = consts.tile([1, 128], BF16)
        bo_bf = consts.tile([1, D], BF16)
        wdum = consts.tile([1, 128], FP16)            # warmup stationary
        rdum = consts.tile([1, 512], FP16)            # warmup moving
        expb = consts.tile([128, 1], FP32)
        vbc = consts.tile([128, HD], FP32)            # bv broadcast over tokens
        obc = consts.tile([128, 2, 512], FP32)        # bo broadcast over tokens
        nc.vector.memset(ones32[:], 1.0)
        nc.vector.memset(onesb[:], 1.0)
        nc.vector.memset(wdum[:], 0.0)
        nc.vector.memset(rdum[:], 0.0)
        # identity for PE transposes: ident[p, f] = (f == p)
        nc.gpsimd.iota(it_f[:], [[0, 1], [1, 128]], base=0, channel_multiplier=0,
                       allow_small_or_imprecise_dtypes=True)
        nc.gpsimd.iota(it_p[:], [[0, 1], [1, 1]], base=0, channel_multiplier=1,
                       allow_small_or_imprecise_dtypes=True)
        nc.vector.tensor_scalar(ident[:], it_f[:], it_p[:], None, ALU.is_equal)
        nc.vector.tensor_copy(identh[:], ident[:])
        nc.vector.memset(expb[:], EXP_BIAS)
        nc.vector.memset(vh[:, :, :, 64:66], DEN_SCALE)

        # ---------------- DRAM staging (fp16 casts) ----------------
        # One DRAM tile per transfer: the tile scheduler tracks DRAM tiles at
        # whole-tile granularity, so a shared staging buffer would serialize
        # every cast against every transpose.  Rotating 256-token x stages
        # throttle the cast stream via real WAR dependencies -- cast i must
        # wait for transpose i-4 -- which forces the scheduler to interleave
        # transposes with casts on the shared DMA device instead of running
        # every cast first.
        xstg = [dram.tile([X_CHUNK, D], FP16, name=f"xst{i}") for i in range(N_STAGES)]
        wo_bf = dram.tile([D, HD], FP16)

        # De-interleave Wqk rows: q rows are 2c, k rows are 2c+1.
        wqk_r = wqk.rearrange("(c s) d -> s c d", s=2)
        bqk_r = bqk.rearrange("(c s) -> s c", s=2)

        # bias loads ride the ACT HWDGE ring (cheap, off the SP transpose ring)
        for cb in range(2):
            nc.scalar.dma_start(bq_sb[:, cb, :], bqk_r[0, cb * 128:(cb + 1) * 128])
            nc.scalar.dma_start(bk_sb[:, cb, :], bqk_r[1, cb * 128:(cb + 1) * 128])
        nc.scalar.dma_start(bv_row[0:1, :], bv[:])
        nc.scalar.dma_start(bo_row[0:1, :], bo[:])

        # ---------------- supply: x casts + xbar transposes, W direct -------
        def x_sup(ci):
            t0, t1 = ci * X_CHUNK, (ci + 1) * X_CHUNK
            st = xstg[ci % N_STAGES]
            nc.gpsimd.dma_start(st[:, :], x[t0:t1, :])
            nc.sync.dma_start(xt[:, :, t0:t1], st[:, :], transpose=True)

        nch = N // X_CHUNK
        x_sup(0)
        nc.scalar.dma_start(wq32[:, 0, :], wqk_r[0, 0:128])
        nc.scalar.dma_start(wq32[:, 1, :], wqk_r[0, 128:256])
        x_sup(1)
        nc.scalar.dma_start(wk32[:, 0, :], wqk_r[1, 0:128])
        nc.scalar.dma_start(wk32[:, 1, :], wqk_r[1, 128:256])
        if nch > 4:
            x_sup(2)
            x_sup(3)
        nc.scalar.dma_start(wv32[:, 0, :], wv[0:128, :])
        nc.scalar.dma_start(wv32[:, 1, :], wv[128:256, :])
        for ci in range(min(4, nch // 2), nch):
            x_sup(ci)
        prio0 = tc.cur_priority
        if WO_DEPRIO:
            # Make the Wo supply look late to the tile scheduler so it cannot
            # jump the shared DMA device ahead of the x/k/v supply.
            tc.cur_priority += 100000
        nc.gpsimd.dma_start(wo_bf[:], wo[:])
        for g in range(2):
            # Wo^T must match yT's c-layout (c = cc*128 + p): per-chunk.
            cs = slice(g * 128, (g + 1) * 128)
            nc.sync.dma_start(woT[:, g, :], wo_bf[:, cs], transpose=True)
        if WO_DEPRIO:
            tc.cur_priority = prio0

        # ---------------- PE warmup ----------------
        # Keep the PE stream dense from ~1us so the p-state ramp is spent on
        # throwaway work, and the first real chains run at full clock.
        for _ in range(N_WARM):
            pw = ps.tile([128, 512], FP32, tag="sp")
            nc.tensor.matmul(pw[:], lhsT=wdum[:], rhs=rdum[:],
                             start=True, stop=True)

        def bias_broadcasts():
            nc.vector.tensor_copy(bo_bf[:], bo_row[:])
            pb = ps.tile([128, 512], FP32, tag="sp")
            nc.tensor.matmul(pb[:, 0:HD], lhsT=ones32[:], rhs=bv_row[:],
                             start=True, stop=True)
            nc.vector.tensor_copy(vbc[:], pb[:, 0:HD])
            for ob in range(2):
                pb2 = ps.tile([128, 512], FP32, tag="sp")
                nc.tensor.matmul(pb2[:], lhsT=ones32[:],
                                 rhs=bo_row[:, ob * 512:(ob + 1) * 512],
                                 start=True, stop=True)
                nc.vector.tensor_copy(obc[:, ob, :], pb2[:])

        # ---------------- W^T via PE transpose ----------------
        # 4 transposed [128,128] d-chunks per PSUM slab, one strided DVE
        # evacuation (with the fp16 cast) per slab.
        def w_transpose(w32, dstT):
            for cb in range(2):
                for g in range(2):
                    tp = ps.tile([128, 512], FP32, tag="sp")
                    for j in range(4):
                        dc = g * 4 + j
                        nc.tensor.transpose(
                            tp[:, j * 128:(j + 1) * 128],
                            w32[:, cb, dc * 128:(dc + 1) * 128],
                            ident[:],
                        )
                    nc.vector.tensor_copy(
                        dstT[:, g * 4:(g + 1) * 4, cb * 128:(cb + 1) * 128],
                        tp[:].rearrange("p (dc c) -> p dc c", dc=4),
                    )

        # ---------------- projection chains ----------------
        def qk_chain(wT, b_sb, dstT, cb, t0, t1):
            pp = ps.tile([128, t1 - t0], FP32, tag="sp")
            for dc in range(DC):
                nc.tensor.matmul(
                    pp[:],
                    lhsT=wT[:, dc, cb * 128:(cb + 1) * 128],
                    rhs=xt[:, dc, t0:t1],
                    start=(dc == 0),
                    stop=(dc == DC - 1),
                )
            # evacuate + per-partition bias + cast to fp16 on DVE
            nc.vector.tensor_scalar(
                dstT[:, cb, t0:t1], pp[:], b_sb[:, cb, :], None, ALU.add,
            )

        def v_chain(jb):
            pv = ps.tile([128, 512], FP32, tag="sp")
            for dc in range(DC):
                nc.tensor.matmul(
                    pv[:, 0:HD],
                    lhsT=xt[:, dc, jb * 128:(jb + 1) * 128],
                    rhs=wvT[:, dc, :],
                    start=(dc == 0),
                    stop=(dc == DC - 1),
                )
            nc.vector.tensor_tensor(vh[:, jb, :, 0:64], pv[:, 0:HD], vbc[:], ALU.add)

        # ---------------- attention sweep machinery ----------------
        # One sweep = (ib, cc): i-block of 512 tokens x head pair (2cc, 2cc+1).
        # S is the classic [j, i] orientation; AV is transposed: av[i, 0:65]
        # accumulates over all 16 j-blocks with the exp tile as the stationary
        # operand, so each AV matmul is charged only 65 output columns.
        def make_sweep(cc, isl):
            avA = psav.tile([128, 4, 128], FP32, tag="avA", bufs=1, name="avA")
            avB = psav.tile([128, 4, 128], FP32, tag="avB", bufs=1, name="avB")
            st = {"avA": avA, "avB": avB, "exs": {}}

            def emit_S(jb):
                jsl = slice(jb * 128, (jb + 1) * 128)
                sp = ps.tile([128, 1024], FP32, tag="sp")
                nc.tensor.matmul(
                    sp[:, 0:512],
                    lhsT=kT[0:64, cc, jsl], rhs=qT[0:64, cc, isl],
                    start=True, stop=True,
                )
                nc.tensor.matmul(
                    sp[:, 512:1024],
                    lhsT=kT[64:128, cc, jsl], rhs=qT[64:128, cc, isl],
                    start=True, stop=True,
                )
                ex = exps_pool.tile([128, 1024], BF16)
                nc.scalar.activation(ex[:], sp[:], AF.Exp, bias=expb[:])
                st["exs"][jb] = ex

            def emit_AV(jb):
                ex = st["exs"].pop(jb)
                last = (jb == NJB - 1)
                for t in range(4):
                    tsl = slice(t * 128, (t + 1) * 128)
                    # start=True clears has_written for the WHOLE PSUM bank,
                    # so only the first region of the first j-block may carry
                    # it; the other t-regions of jb 0 write onto the cleared
                    # bank with start=False (has_written=0 -> plain write).
                    first = (jb == 0) and (t == 0)
                    nc.tensor.matmul(
                        st["avA"][:, t, 0:65],
                        lhsT=ex[:, tsl], rhs=vh[:, jb, 2 * cc, 0:65],
                        start=first, stop=last, skip_group_check=True,
                    )
                    nc.tensor.matmul(
                        st["avB"][:, t, 0:65],
                        lhsT=ex[:, 512 + t * 128:512 + (t + 1) * 128],
                        rhs=vh[:, jb, 2 * cc + 1, 0:65],
                        start=first, stop=last, skip_group_check=True,
                    )

            st["S"] = emit_S
            st["AV"] = emit_AV
            return st

        # Normalize at sweep end (DVE): rec = 1/(16*sum exp) is one column of
        # the transposed av, so a per-partition reciprocal + tensor_scalar
        # multiply normalizes and casts to fp16 in one op per (head, i128).
        def normalize(st):
            rec = avn_pool.tile([128, 4, 2], FP32, tag="rec")
            nc.vector.reciprocal(rec[:, :, 0:1], st["avA"][:, :, 64:65])
            nc.vector.reciprocal(rec[:, :, 1:2], st["avB"][:, :, 64:65])
            avnA = avn_pool.tile([128, 4, E], FP16, tag="avnA")
            avnB = avn_pool.tile([128, 4, E], FP16, tag="avnB")
            for t in range(4):
                nc.vector.tensor_scalar(
                    avnA[:, t, :], st["avA"][:, t, 0:E], rec[:, t, 0:1],
                    None, ALU.mult,
                )
                nc.vector.tensor_scalar(
                    avnB[:, t, :], st["avB"][:, t, 0:E], rec[:, t, 1:2],
                    None, ALU.mult,
                )
            return avnA, avnB

        # Deferred into the next sweep's fillers: PE transposes restore the
        # [c, i] layout for the out-projection (head pair stacked on
        # partitions, two per [128, 128] PSUM tile), one DVE copy each.
        def make_yt_tail(cc, ib, avnA, avnB, t):
            def yt_tail():
                tp = ps.tile([128, 128], FP16, tag="sp")
                nc.tensor.transpose(tp[0:64, :], avnA[:, t, :], identh[:])
                nc.tensor.transpose(tp[64:128, :], avnB[:, t, :], identh[:])
                isl = slice(ib * 512 + t * 128, ib * 512 + (t + 1) * 128)
                nc.vector.tensor_copy(yT[:, cc, isl], tp[:])
            return yt_tail

        # Sweep (ib0, cc0) state: its S/exp/AV stream is interleaved into the
        # projection phase below so the ACT exp pipe starts ~40us earlier.
        s00 = make_sweep(0, slice(0, 512))

        def s00_block(b):
            for jb in range(4 * b, 4 * b + 4):
                s00["S"](jb)
                if jb >= 2:
                    s00["AV"](jb - 2)

        # W transposes and q(ib=0)/k/v chains in x-arrival order; the Wk/Wv
        # fp32 loads are emitted at their need sites so the tile scheduler
        # cannot let them jump the DMA device ahead of the early x chunks.
        w_transpose(wq32, wqT)
        for t0, t1 in ((0, 256), (256, 512)):
            qk_chain(wqT, bq_sb, qT, 0, t0, t1)
            qk_chain(wqT, bq_sb, qT, 1, t0, t1)
        w_transpose(wk32, wkT)
        qk_chain(wkT, bk_sb, kT, 0, 0, 512)
        qk_chain(wkT, bk_sb, kT, 1, 0, 512)
        bias_broadcasts()
        w_transpose(wv32, wvT)
        for jb in range(4):
            v_chain(jb)
        s00_block(0)
        for tb in range(1, 4):
            ts0, ts1 = tb * 512, (tb + 1) * 512
            qk_chain(wkT, bk_sb, kT, 0, ts0, ts1)
            qk_chain(wkT, bk_sb, kT, 1, ts0, ts1)
            for jb in range(tb * 4, (tb + 1) * 4):
                v_chain(jb)
            s00_block(tb)

        # ---------------- attention + out-projection ----------------
        def oproj_tile(it, ob, tail=False):
            # out[i, do] = sum_c yT[c, i] * WoT[c, do] + bo[do]
            tsl = slice(it * 128, (it + 1) * 128)
            po = ps.tile([128, 512], FP32, tag="sp")
            osl = slice(ob * 512, (ob + 1) * 512)
            for cc2 in range(2):
                nc.tensor.matmul(
                    po[:],
                    lhsT=yT[:, cc2, tsl],
                    rhs=woT[:, cc2, osl],
                    start=(cc2 == 0), stop=(cc2 == 1) and not tail,
                )
            osb = osb_pool.tile([128, 512], FP32)
            if tail:
                # final drain is DVE-bound: fold bo into the accumulation via
                # a K=1 ones-row matmul and evacuate on the then-idle ACT
                # engine instead of adding obc on DVE.
                nc.tensor.matmul(po[:], lhsT=onesb[:], rhs=bo_bf[:, osl],
                                 start=False, stop=True)
                nc.scalar.activation(osb[:], po[:], AF.Copy)
            else:
                nc.vector.tensor_tensor(osb[:], po[:], obc[:, ob, :], ALU.add)
            nc.sync.dma_start(out_ap[tsl, osl], osb[:])

        # q-chains are split into two ~4-matmul filler quanta so they slot
        # between S pairs without draining the exp stream.  Parts A and B of
        # one chain must land on consecutive filler slots: with 3 sp slabs
        # the chain's accumulator survives exactly two interleaved S-pair
        # allocations.
        def q_fill_pair(ib2, cb):
            t0, t1 = ib2 * 512, (ib2 + 1) * 512
            state = {}

            def part_a():
                pp = ps.tile([128, 512], FP32, tag="sp", name="qf")
                state["pp"] = pp
                for dc in range(4):
                    nc.tensor.matmul(
                        pp[:],
                        lhsT=wqT[:, dc, cb * 128:(cb + 1) * 128],
                        rhs=xt[:, dc, t0:t1],
                        start=(dc == 0), stop=False,
                    )

            def part_b():
                pp = state["pp"]
                for dc in range(4, DC):
                    nc.tensor.matmul(
                        pp[:],
                        lhsT=wqT[:, dc, cb * 128:(cb + 1) * 128],
                        rhs=xt[:, dc, t0:t1],
                        start=False, stop=(dc == DC - 1),
                    )
                nc.vector.tensor_scalar(
                    qT[:, cb, t0:t1], pp[:], bq_sb[:, cb, :], None, ALU.add,
                )
            return [part_a, part_b]

        # Per-sweep filler queues (sweep s = ib*2 + cc).  The deferred yt
        # tails of sweep s-1 are inserted inside the loop.  Sweep 0's S/exp/AV
        # work is interleaved into the projection phase (it has no filler
        # slots), so the q-chain fillers all land in sweeps 1-5.
        sweep_fillers = {s: [] for s in range(2 * NIB + 1)}
        sweep_fillers[1] = q_fill_pair(1, 0) + q_fill_pair(1, 1)
        sweep_fillers[2] = q_fill_pair(2, 0)
        sweep_fillers[3] = q_fill_pair(2, 1)
        sweep_fillers[4] = q_fill_pair(3, 0)
        sweep_fillers[5] = q_fill_pair(3, 1)
        for ib in range(1, NIB):
            prev = ib - 1
            tiles = [(prev * 4 + t, o) for t in range(4) for o in range(2)]
            for k in range(2):
                sweep_fillers[2 * ib + k] += [
                    (lambda it=it, ob=ob: oproj_tile(it, ob))
                    for it, ob in tiles[k * 4:(k + 1) * 4]
                ]
        sweep_fillers[2 * NIB] = [
            (lambda it=12 + t, ob=o: oproj_tile(it, ob, tail=True))
            for t in range(4) for o in range(2)
        ]

        for ib in range(NIB):
            isl = slice(ib * 512, (ib + 1) * 512)
            for cc in range(2):          # head pair (2*cc, 2*cc+1)
                s_idx = ib * 2 + cc
                fl = sweep_fillers[s_idx]
                fi = 0
                if s_idx == 0:
                    # S/exp and AV(0..13) were interleaved into the
                    # projection phase; only the pipeline drain remains.
                    st = s00
                    st["AV"](NJB - 2)
                    st["AV"](NJB - 1)
                else:
                    st = make_sweep(cc, isl)
                    # software pipeline: S runs two j-blocks ahead of AV, so
                    # each exp has a full extra period of ACT margin.
                    st["S"](0)
                    st["S"](1)
                    for jb in range(2, NJB):
                        st["S"](jb)
                        st["AV"](jb - 2)
                        if jb % FILL_EVERY == 1 and fi < len(fl):
                            fl[fi]()
                            fi += 1
                    st["AV"](NJB - 2)
                    st["AV"](NJB - 1)
                while fi < len(fl):
                    fl[fi]()
                    fi += 1

                avnA, avnB = normalize(st)
                tails = [make_yt_tail(cc, ib, avnA, avnB, t) for t in range(4)]
                nxt = sweep_fillers[s_idx + 1]
                pos = min(NT_POS, len(nxt))
                nxt[pos:pos] = tails

        # tail: last yt transposes first, then out-projection of the last
        # i-block.  Data deps (semaphores) order the yT writes before their
        # oproj readers.
        tail = sweep_fillers[2 * NIB]
        for f in tail:
            f()


_CACHE: dict = {}


def _get_compiled():
    key = "nc"
    if key in _CACHE:
        return _CACHE[key]
    nc = bacc.Bacc(
        "TRN2", target_bir_lowering=False, debug=False, num_devices=NCORES,
    )
    x = nc.dram_tensor("x", (N, D), FP32, kind="ExternalInput").ap()
    wqk = nc.dram_tensor("Wqk", (2 * HD, D), FP32, kind="ExternalInput").ap()
    bqk = nc.dram_tensor("bqk", (2 * HD,), FP32, kind="ExternalInput").ap()
    wv = nc.dram_tensor("Wv", (HD, D), FP32, kind="ExternalInput").ap()
    bv = nc.dram_tensor("bv", (HD,), FP32, kind="ExternalInput").ap()
    wo = nc.dram_tensor("Wo", (D, HD), FP32, kind="ExternalInput").ap()
    bo = nc.dram_tensor("bo", (D,), FP32, kind="ExternalInput").ap()
    out = nc.dram_tensor("out", (N, D), FP32, kind="ExternalOutput").ap()

    with tile.TileContext(nc) as tc:
        _build_kernel(nc, tc, out, x, wqk, bqk, wv, bv, wo, bo)
    nc.compile()
    _CACHE[key] = nc
    return nc


def run_cores(in_maps, trace=False, **kw):
    nc = _get_compiled()
    return bass_utils.run_bass_kernel_spmd(
        nc, in_maps, core_ids=list(range(NCORES)), trace=trace, **kw
    )


def kernel(x, Wqk, bqk, Wv, bv, Wo, bo):
    x = np.asarray(x, dtype=np.float32)
    in_maps = [
        {
            "x": np.ascontiguousarray(x[c]),
            "Wqk": np.asarray(Wqk, np.float32),
            "bqk": np.asarray(bqk, np.float32),
            "Wv": np.asarray(Wv, np.float32),
            "bv": np.asarray(bv, np.float32),
            "Wo": np.asarray(Wo, np.float32),
            "bo": np.asarray(bo, np.float32),
        }
        for c in range(NCORES)
    ]
    # The axon tunnel occasionally returns a glitched execution (transient
    # non-finite garbage); retry a couple of times in that case.
    for _attempt in range(3):
        res = run_cores(in_maps)
        out = np.stack([res.results[c]["out"] for c in range(NCORES)], axis=0)
        if np.isfinite(out).all():
            break
    return out
